# revision 63
# baseline (speedup 1.0000x reference)
"""Trainium2 Bass kernel for DeepBayesianDKVMN (nn_DeepBayesianDKVMN_39857296507058).

Math restructuring
------------------
The reference's sequential Bayesian-write scan is *linear* in the memory
state: the per-step precision/evidence increments depend only on step-t
inputs, never on the evolving state.  So the scan collapses to exclusive
cumulative sums over time, and everything else is batch-parallel:

  - front end: exp(q_table[q_ids] @ q2k_w @ key_embeds.T) folds into ONE
    fp16 table gather.  The table packs [exp(sim) | exp(sim)^2] into 256B
    rows, so a single descriptor per (b,t) brings both powers needed by
    the write-aggregation stats.
  - the per-(t,m) softmax bias factor eb = exp(bias) FACTORS OUT of every
    batch contraction: the H stats use raw es with an eb / eb^2
    post-scale on the tiny [M, S, 6] aggregate, and the m-contractions
    (softmax normalizer z, prediction dot) absorb eb into their per-(t,m)
    right-hand-side vectors.
  - each gathered chunk's es half is PE-transposed (f16 PSUM staging +
    scalar-engine copy) into a resident esT with m on partitions;
    z[b,t] = sum_m es*eb and num[b,t] = sum_m es*(eb*g) are then per-t PE
    matvecs (lhsT = esT_t, rhs = eb / eb*g columns) - no broadcasts, no
    big vector reduces, no DMA transpose.
  - per-(b,t) evidence reduces to THREE [S,M] batch sums via tiny per-t
    PE matmuls (contraction over the 128 batch rows on partitions): the
    rank-4 evidence combination folds into ONE host-precomputed feature
    (everything downstream is linear), and the 1/Z softmax normalizations
    fold into the per-t feature matrix.
  - two fp16 AllReduces over [S/2, M, 3] stat halves - the first overlaps
    the remaining gathers, and the prefix-sum structure lets the whole
    first half of the epilogue (cumsums, read vector g, prediction
    matvecs) run under phase A too; exclusive cumsums via
    tensor_tensor_scan with cross-half initial-value chaining;
    preds = zr * (es . (eb*g)) + pred_b.

Sharding: batch 1024 -> 128 rows per core across 8 cores (data parallel),
as the sharding hint suggests; the all-reduce is the per-slot aggregated
evidence/precision, shrunk by the rank-4 trick.
"""

import numpy as np
from contextlib import ExitStack

import concourse.bass as bass
import concourse.tile as tile
from concourse import bacc, mybir
from concourse.bass_utils import run_bass_kernel_spmd

# problem dims (hardcoded per spec)
B, S, M, K, V, E, NQ, C = 1024, 512, 64, 64, 128, 64, 10000, 4
NCORES = 8
BL = B // NCORES            # 128 batch rows per core
TC = 32                     # timestep chunk
NCH = S // TC               # 16 chunks
NIDX = BL * TC              # gather indices per chunk = 4096
# tapered chunk schedule: short first chunk (earlier first gather), short
# last chunks (shorter post-gather serial tail)
CHUNKS = [16, 16] + [32] * 15
NSEG = 2                    # stat all-reduce segments (collectives)
SEG_ENDS = [256, 512]       # segment boundaries (chunk-aligned)
M2 = 2 * M                  # packed row: [es | es^2]
F32 = mybir.dt.float32
F16 = mybir.dt.float16
I16 = mybir.dt.int16
GSCALE = 1024.0            # fp16 pre-scale for the tiny g values
ALU = mybir.AluOpType
AXT = mybir.AxisListType
ACTF = mybir.ActivationFunctionType

_CACHE = {}


def _build(single_core=False, reps=1):
    """Build the Bass program once per process.

    reps > 1 unrolls the whole algorithm multiple times (used only for
    wall-clock slope timing; outputs are just overwritten).
    """
    nc = bacc.Bacc("TRN2", target_bir_lowering=False, debug=False,
                   num_devices=1 if single_core else NCORES)

    t_esim = nc.dram_tensor("esim", [NQ + 1, M2], F16, kind="ExternalInput")
    t_idx = nc.dram_tensor("idx16", [128, NCH * NIDX // 16], I16,
                           kind="ExternalInput")
    t_ftab = nc.dram_tensor("ftab", [BL, S, 2], F16, kind="ExternalInput")
    t_ebdup = nc.dram_tensor("ebdup", [128, S], F16, kind="ExternalInput")
    t_ebm = nc.dram_tensor("ebm", [M, S], F16, kind="ExternalInput")
    t_ebm2 = nc.dram_tensor("ebm2", [M, S], F16, kind="ExternalInput")
    t_ident = nc.dram_tensor("ident", [128, 128], F16, kind="ExternalInput")
    t_dup = nc.dram_tensor("dup64", [M, 128], F16, kind="ExternalInput")
    t_sc = nc.dram_tensor("scal", [M, 8], F32, kind="ExternalInput")
    t_pb = nc.dram_tensor("pb", [BL, 1], F32, kind="ExternalInput")
    t_preds = nc.dram_tensor("preds", [BL, S], F32, kind="ExternalOutput")

    with tile.TileContext(nc) as tc, ExitStack() as outer_ctx:
        for _rep in range(reps):
            _build_body(nc, tc, outer_ctx, single_core, t_esim, t_idx,
                        t_ftab, t_ebdup, t_ebm, t_ebm2, t_ident, t_dup,
                        t_sc, t_pb, t_preds)
    nc.compile()
    return nc


def _build_body(nc, tc, _outer, single_core, t_esim, t_idx, t_ftab, t_ebdup,
                t_ebm, t_ebm2, t_ident, t_dup, t_sc, t_pb, t_preds):
    with ExitStack() as ctx:
        cpool = ctx.enter_context(tc.tile_pool(name="const", bufs=1))
        dpool = ctx.enter_context(tc.tile_pool(name="dram", bufs=1,
                                               space="DRAM"))
        # internal DRAM (quarters, so the early AllReduces overlap phase A)
        seg0 = [0] + SEG_ENDS[:-1]
        d_hin = [dpool.tile([M, (SEG_ENDS[h] - seg0[h]) * 3], F16,
                            name=f"d_hin{h}") for h in range(NSEG)]
        d_hout = [dpool.tile([M, (SEG_ENDS[h] - seg0[h]) * 3], F16,
                             addr_space="Shared", name=f"d_hout{h}")
                  for h in range(NSEG)]

        # resident SBUF; chunk-0 gather indices land first so the first
        # gather starts as early as possible
        I0 = BL * CHUNKS[0] // 16
        idx_sb = cpool.tile([128, NCH * NIDX // 16], I16)
        nc.sync.dma_start(idx_sb[:, 0:I0], t_idx.ap()[:, 0:I0])
        nc.sync.dma_start(idx_sb[:, I0:], t_idx.ap()[:, I0:])
        ftab_sb = cpool.tile([BL, S, 2], F16)
        nc.sync.dma_start(ftab_sb[:], t_ftab.ap())
        ebdup_sb = cpool.tile([128, S], F16)
        nc.sync.dma_start(ebdup_sb[:], t_ebdup.ap())
        ebm_sb = cpool.tile([M, S], F16)
        nc.sync.dma_start(ebm_sb[:], t_ebm.ap())
        ebm2_sb = cpool.tile([M, S], F16)
        nc.sync.dma_start(ebm2_sb[:], t_ebm2.ap())
        ident_sb = cpool.tile([128, 128], F16)
        nc.sync.dma_start(ident_sb[:], t_ident.ap())
        dup_sb = cpool.tile([M, 128], F16)
        nc.sync.dma_start(dup_sb[:], t_dup.ap())
        sc_sb = cpool.tile([M, 8], F32)
        nc.sync.dma_start(sc_sb[:], t_sc.ap())
        pb_sb = cpool.tile([BL, 1], F32)
        nc.sync.dma_start(pb_sb[:], t_pb.ap())
        zr = cpool.tile([BL, S], F32)
        hbuf = cpool.tile([M, S * 3], F16)
        # transposed es, resident: esT[(t%2)*64 + m, t//2, b] = es[b, t, m]
        esT = cpool.tile([128, S // 2, 128], F16)

        # ---------------- phase A: gather [es|es^2], PE-transpose es,
        # z matvecs, softmax stats + H matmuls.  Tapered chunk schedule:
        # small first chunk so the first gather starts sooner, small last
        # chunks so the post-gather serial tail is short.
        with ExitStack() as actx:
            apool = actx.enter_context(tc.tile_pool(name="pha", bufs=1))
            spool = actx.enter_context(tc.tile_pool(name="phs", bufs=2))
            pspool = actx.enter_context(
                tc.tile_pool(name="php", bufs=2, space="PSUM"))
            ptpool = actx.enter_context(
                tc.tile_pool(name="pht", bufs=1, space="PSUM"))
            t0 = 0
            for c, tcs in enumerate(CHUNKS):
                ts = slice(t0, t0 + tcs)
                nidx = BL * tcs
                i0 = t0 * BL // 16
                getag, gebufs = ("ge32", 4) if tcs == 32 else ("ge16", 2)
                ge = apool.tile([BL, tcs * M2], F16, tag=getag, bufs=gebufs)
                ge3 = ge[:].rearrange("p (a b) -> p a b", b=M2)
                # the last chunk's gather is split in two so its compute
                # pipeline starts halfway through (shorter serial tail)
                ngath = 2 if c == len(CHUNKS) - 1 else 1
                for gs in range(ngath):
                    nsub = nidx // ngath
                    tsub = tcs // ngath
                    nc.gpsimd.dma_gather(
                        out_ap=ge3[:, gs * tsub:(gs + 1) * tsub, :],
                        in_ap=t_esim.ap(),
                        idxs_ap=idx_sb[:, i0 + gs * nsub // 16:
                                       i0 + (gs + 1) * nsub // 16],
                        num_idxs=nsub,
                        num_idxs_reg=nsub,
                        elem_size=M2,
                        single_packet=False,
                    )
                es3 = ge3[:, :, 0:M]
                es23 = ge3[:, :, M:M2]
                # PE-transpose the es half into f16 PSUM (16 timesteps per
                # 1-bank staging tile), then scalar engine copies to the
                # resident esT (keeps the DMA engines free for the gathers)
                for sb in range(tcs // 16):
                    pst = ptpool.tile([128, 8, 128], F16, tag="pst", bufs=2)
                    for tt in range(16):
                        t = sb * 16 + tt
                        nc.tensor.transpose(
                            pst[(t % 2) * M:(t % 2 + 1) * M, tt // 2, :],
                            ge3[:, t, 0:M], ident_sb[:])
                    nc.scalar.copy(
                        esT[:, (t0 + sb * 16) // 2:(t0 + sb * 16) // 2 + 8,
                            :], pst[:])
                # z[b,t] = sum_m es * eb  via per-t matvecs on the PE;
                # zp and hp share one 1-bank PSUM tile
                zh = pspool.tile([128, 128], F32, tag="zh")
                zp = zh[:, 0:tcs]
                hp = zh[0:M, 32:32 + tcs * 3]
                for t in range(tcs):
                    tg = t0 + t
                    half = tg % 2
                    nc.tensor.matmul(
                        zp[:, t:t + 1],
                        lhsT=esT[half * M:(half + 1) * M, tg // 2, :],
                        rhs=ebdup_sb[half * M:(half + 1) * M, tg:tg + 1],
                        start=True, stop=True)
                zrc = zr[:, ts]
                nc.vector.reciprocal(zrc, zp)
                zr2 = spool.tile([BL, tcs], F32, tag=f"zr2{tcs}")
                nc.vector.tensor_tensor(zr2[:], zrc, zrc, ALU.mult)
                # per-t feature matrix: [zr, pr*zr, F0*zr2] (F0 combines the
                # four es^2 features on the host - everything downstream of
                # the H stats is linear in them)
                fp = spool.tile([BL, tcs, 3], F16, tag=f"fp{tcs}")
                nc.vector.tensor_copy(fp[:, :, 0:1], zrc.unsqueeze(2))
                nc.vector.tensor_tensor(fp[:, :, 1:2], ftab_sb[:, ts, 0:1],
                                        zrc.unsqueeze(2), ALU.mult)
                nc.vector.tensor_tensor(fp[:, :, 2:3], ftab_sb[:, ts, 1:2],
                                        zr2[:].unsqueeze(2), ALU.mult)
                # H stats: contraction over batch on the PE, per t
                for t in range(tcs):
                    nc.tensor.matmul(hp[:, t * 3:t * 3 + 2], lhsT=es3[:, t, :],
                                     rhs=fp[:, t, 0:2], start=True, stop=True)
                    nc.tensor.matmul(hp[:, t * 3 + 2:t * 3 + 3],
                                     lhsT=es23[:, t, :], rhs=fp[:, t, 2:3],
                                     start=True, stop=True)
                nc.vector.tensor_copy(hbuf[:, t0 * 3:(t0 + tcs) * 3], hp)
                t0 += tcs
                # kick off each quarter's AllReduce as soon as its stats
                # are complete so the early ones overlap phase A
                if t0 in SEG_ENDS:
                    h = SEG_ENDS.index(t0)
                    hsl = slice(seg0[h] * 3, t0 * 3)
                    nc.sync.dma_start(d_hin[h][:], hbuf[:, hsl])
                    if single_core:
                        nc.sync.dma_start(d_hout[h][:], d_hin[h][:])
                    else:
                        nc.gpsimd.collective_compute(
                            "AllReduce", ALU.add,
                            replica_groups=[list(range(NCORES))],
                            ins=[d_hin[h][:].opt()],
                            outs=[d_hout[h][:].opt()],
                        )

        # ---------------- phases B/C/D, per S/4 quarter: the prefix-sum
        # structure means each quarter (after its AllReduce) is fully
        # computable while later chunks are still gathering, so only the
        # last quarter sits on the serial tail.  cs/(cs+1e-8) == 1 to
        # float precision for the attention column sums here, so cc
        # simplifies to P/B.
        DC = 128
        hs = cpool.tile([M, S, 3], F16)
        gdup = cpool.tile([128, S], F16)
        rtile = cpool.tile([BL, S], F32)
        cpool2 = ctx.enter_context(tc.tile_pool(name="phc", bufs=2))
        psg = ctx.enter_context(tc.tile_pool(name="phg", bufs=2,
                                             space="PSUM"))
        psd = ctx.enter_context(tc.tile_pool(name="phdp", bufs=2,
                                             space="PSUM"))
        ch_prev = ccs_prev = None
        hs_prev = 0
        for h in range(NSEG):
            HS = SEG_ENDS[h] - seg0[h]
            hsl = slice(seg0[h], SEG_ENDS[h])
            hsv = hs[:, hsl, :]
            nc.sync.dma_start(hsv, d_hout[h][:])
            css = cpool2.tile([M, HS], F32, tag="css")
            nc.vector.tensor_tensor(css[:], hsv[:, :, 0], ebm_sb[:, hsl],
                                    ALU.mult)
            rcs = cpool2.tile([M, HS], F32, tag="rcs")
            nc.vector.reciprocal(rcs[:], css[:])
            cc = cpool2.tile([M, HS], F32, tag="cc")
            nc.vector.scalar_tensor_tensor(cc[:], hsv[:, :, 1], 1.0 / B,
                                           ebm_sb[:, hsl], ALU.mult,
                                           ALU.mult)
            sfac2 = cpool2.tile([M, HS], F32, tag="sfac2")
            nc.vector.tensor_tensor(sfac2[:], cc[:], rcs[:], ALU.mult)
            nc.vector.tensor_tensor(sfac2[:], sfac2[:], ebm2_sb[:, hsl],
                                    ALU.mult)
            # num = cumsum_excl(Hc * sfac2) + n0pw
            hck = cpool2.tile([M, HS], F32, tag="hck")
            nc.vector.tensor_tensor(hck[:], hsv[:, :, 2], sfac2[:], ALU.mult)
            ch = cpool2.tile([M, HS + 1], F32, tag="ch")
            if h == 0:
                nc.vector.memset(ch[:, 0:1], 0.0)
                ini_h = 0.0
            else:
                nc.vector.tensor_copy(ch[:, 0:1],
                                      ch_prev[:, hs_prev:hs_prev + 1])
                ini_h = ch_prev[:, hs_prev:hs_prev + 1]
            nc.vector.tensor_tensor_scan(ch[:, 1:HS + 1], hck[:], hck[:],
                                         ini_h, ALU.add, ALU.bypass)
            num = cpool2.tile([M, HS], F32, tag="num")
            nc.vector.tensor_scalar_add(num[:], ch[:, 0:HS], sc_sb[:, 5:6])
            # den = alo + CC_excl ; g = num / den
            ccs = cpool2.tile([M, HS + 1], F32, tag="ccs")
            if h == 0:
                nc.vector.memset(ccs[:, 0:1], 0.0)
                ini_c = 0.0
            else:
                nc.vector.tensor_copy(ccs[:, 0:1],
                                      ccs_prev[:, hs_prev:hs_prev + 1])
                ini_c = ccs_prev[:, hs_prev:hs_prev + 1]
            nc.vector.tensor_tensor_scan(ccs[:, 1:HS + 1], cc[:], cc[:],
                                         ini_c, ALU.add, ALU.bypass)
            den = cpool2.tile([M, HS], F32, tag="den")
            nc.vector.tensor_scalar_add(den[:], ccs[:, 0:HS], sc_sb[:, 4:5])
            rden = cpool2.tile([M, HS], F32, tag="rden")
            nc.vector.reciprocal(rden[:], den[:])
            g = cpool2.tile([M, HS], F32, tag="g")
            nc.vector.tensor_tensor(g[:], num[:], rden[:], ALU.mult)
            ch_prev, ccs_prev, hs_prev = ch, ccs, HS
            # gamma = g * eb * GSCALE (fp16 normal range), duplicated onto
            # both partition halves with a tiny PE matmul
            # (dup_sb[k, i] = [i%64==k])
            gm16 = cpool2.tile([M, HS], F16, tag="gm16")
            nc.vector.scalar_tensor_tensor(gm16[:], g[:], GSCALE,
                                           ebm_sb[:, hsl], ALU.mult,
                                           ALU.mult)
            gp_ps = psg.tile([128, HS], F32, tag="gp_ps")
            nc.tensor.matmul(gp_ps[:], lhsT=dup_sb[:], rhs=gm16[:],
                             start=True, stop=True)
            nc.scalar.copy(gdup[:, hsl], gp_ps[:])
            # phase D for this half: super-chunks of 128 timesteps (one
            # PSUM bank, one fused multiply each)
            for c in range(seg0[h] // DC, SEG_ENDS[h] // DC):
                rp = psd.tile([BL, DC], F32, tag="rp")
                for t in range(DC):
                    tg = c * DC + t
                    half = tg % 2
                    nc.tensor.matmul(
                        rp[:, t:t + 1],
                        lhsT=esT[half * M:(half + 1) * M, tg // 2, :],
                        rhs=gdup[half * M:(half + 1) * M, tg:tg + 1],
                        start=True, stop=True)
                # fold the PSUM read into the zr multiply
                nc.vector.tensor_tensor(rtile[:, c * DC:(c + 1) * DC],
                                        rp[:], zr[:, c * DC:(c + 1) * DC],
                                        ALU.mult)
            nc.vector.tensor_scalar(rtile[:, hsl], rtile[:, hsl],
                                    1.0 / GSCALE, pb_sb[:, 0:1], ALU.mult,
                                    ALU.add)
        # preds out-DMAs last, so they can't head-of-line block the hs
        # landings on the SP sequencer
        for h in range(NSEG):
            hsl = slice(seg0[h], SEG_ENDS[h])
            nc.sync.dma_start(t_preds.ap()[:, hsl], rtile[:, hsl])


def _softplus(x):
    return np.logaddexp(0.0, x)


def _host_prep(inputs):
    """All the cheap host-side precomputation; returns per-core in_maps."""
    q_ids = np.asarray(inputs["q_ids"], np.int64)          # [B, S]
    responses = np.asarray(inputs["responses"], np.int64)  # [B, S]
    q_table = np.asarray(inputs["q_table"], np.float32)
    key_embeds = np.asarray(inputs["key_embeds"], np.float32)
    alpha_mean = np.asarray(inputs["alpha_mean"], np.float32)
    alpha_log_var = np.asarray(inputs["alpha_log_var"], np.float32)
    beta_base = np.asarray(inputs["beta_base"], np.float32)
    beta_offsets = np.asarray(inputs["beta_offsets"], np.float32)
    theta_mean0 = np.asarray(inputs["theta_mean0"], np.float32)
    theta_log_var0 = np.asarray(inputs["theta_log_var0"], np.float32)
    q2k_w = np.asarray(inputs["q2k_w"], np.float32)
    q2k_b = np.asarray(inputs["q2k_b"], np.float32)
    qa_w = np.asarray(inputs["qa_w"], np.float32)
    qa_b = np.asarray(inputs["qa_b"], np.float32)
    qae_w = np.asarray(inputs["qae_w"], np.float32)
    qae_b = np.asarray(inputs["qae_b"], np.float32)
    pred_w = np.asarray(inputs["pred_w"], np.float32)
    pred_b = np.asarray(inputs["pred_b"], np.float32)
    alpha_noise = np.asarray(inputs["alpha_noise"], np.float32)
    beta_noise = np.asarray(inputs["beta_noise"], np.float32)

    # sim table: folds q_table @ q2k_w @ key_embeds.T (+ bias) into a
    # gather; exp is pre-applied and es^2 packed alongside so one 256B
    # descriptor brings both powers
    w_qm = q2k_w @ key_embeds.T                            # [E, M]
    b_m = q2k_b @ key_embeds.T                             # [M]
    es = np.exp(q_table @ w_qm + b_m[None]).astype(np.float32)
    esim = np.empty((NQ + 1, M2), np.float16)
    esim[:, 0:M] = es
    esim[:, M:M2] = es * es

    # per-(t, m) logit bias -> eb = exp(bias)
    alpha = np.exp(alpha_mean[None] + alpha_noise
                   * np.exp(0.5 * alpha_log_var)[None])    # [S, M]
    base = beta_base[None] + beta_noise * 0.1              # [S, M]
    offs = _softplus(beta_offsets)                         # [M, C-1]
    cum = np.concatenate([np.zeros((M, 1), np.float32),
                          np.cumsum(offs, 1)[:, :C - 2]], 1)
    beta_mean = base + cum.mean(1)[None]
    diff_sim = np.exp(-0.5 * beta_mean ** 2)
    ebt = np.exp(0.3 * alpha + 0.2 * diff_sim).astype(np.float32)  # [S, M]
    ebT = np.ascontiguousarray(ebt.T)                      # [M, S]
    ebdup = np.concatenate([ebT, ebT], axis=0).astype(np.float16)  # [128, S]
    ebm = ebT.astype(np.float16)
    ebm2 = (ebT * ebT).astype(np.float16)

    ident = np.eye(128, dtype=np.float16)
    dup64 = np.concatenate([np.eye(M), np.eye(M)], axis=1).astype(np.float16)

    # evidence scalars per (b, t)
    rn = responses.astype(np.float32) / (C - 1)
    p = np.clip(rn, 0.01, 0.99)
    ae = np.log(p) - np.log1p(-p)
    pr = 0.5 + np.abs(rn - 0.5) * 2.0
    q01 = q_ids.astype(np.float32) / NQ

    # rank-4 decomposition of comb over V, folded into ONE feature F0
    w0v = qa_w[0] @ qae_w
    w1v = qa_w[1] @ qae_w
    bv = qa_b @ qae_w + qae_b
    pw = pred_w[:, 0]
    gp = 0.5 * np.array([w0v @ pw, w1v @ pw, bv @ pw, pw.sum()], np.float32)
    f0 = gp[0] * q01 + gp[1] * rn + gp[2] + gp[3] * ae     # [B, S]

    alo = np.exp(-theta_log_var0[:, 0])                    # [M]
    n0pw = alo * (theta_mean0 @ pw)                        # [M]
    sc = np.zeros((M, 8), np.float32)
    sc[:, 4] = alo
    sc[:, 5] = n0pw

    pb = np.full((BL, 1), float(pred_b[0]), np.float32)

    in_maps = []
    for core in range(NCORES):
        bs = slice(core * BL, (core + 1) * BL)
        qs = q_ids[bs]                                     # [128, S]
        # gather indices, chunk-major, wrapped in 16 partitions
        blocks = []
        t0 = 0
        for tcs in CHUNKS:
            flat = qs[:, t0:t0 + tcs].T.reshape(-1)          # t-major
            w16 = flat.reshape(BL * tcs // 16, 16).T         # [16, nidx/16]
            blocks.append(np.tile(w16, (8, 1)))
            t0 += tcs
        idx16 = np.concatenate(blocks, axis=1).astype(np.int16)

        ftab = np.empty((BL, S, 2), np.float16)
        ftab[:, :, 0] = pr[bs]
        ftab[:, :, 1] = f0[bs]

        in_maps.append({
            "esim": esim,
            "idx16": idx16,
            "ftab": ftab,
            "ebdup": ebdup,
            "ebm": ebm,
            "ebm2": ebm2,
            "ident": ident,
            "dup64": dup64,
            "scal": sc,
            "pb": pb,
        })
    return in_maps


def _run(in_maps, **kw):
    if "nc" not in _CACHE:
        _CACHE["nc"] = _build()
    res = run_bass_kernel_spmd(_CACHE["nc"], in_maps,
                               core_ids=list(range(NCORES)), **kw)
    preds = np.concatenate([res.results[c]["preds"] for c in range(NCORES)],
                           axis=0)
    return preds.astype(np.float32), res


def kernel(**inputs) -> np.ndarray:
    return _run(_host_prep(inputs))[0]


if __name__ == "__main__":
    pass


# revision 67
# speedup vs baseline: 1.0032x; 1.0032x over previous
"""Trainium2 Bass kernel for DeepBayesianDKVMN (nn_DeepBayesianDKVMN_39857296507058).

Math restructuring
------------------
The reference's sequential Bayesian-write scan is *linear* in the memory
state: the per-step precision/evidence increments depend only on step-t
inputs, never on the evolving state.  So the scan collapses to exclusive
cumulative sums over time, and everything else is batch-parallel:

  - front end: exp(q_table[q_ids] @ q2k_w @ key_embeds.T) folds into ONE
    fp16 table gather.  The table packs [exp(sim) | exp(sim)^2] into 256B
    rows, so a single descriptor per (b,t) brings both powers needed by
    the write-aggregation stats.
  - the per-(t,m) softmax bias factor eb = exp(bias) FACTORS OUT of every
    batch contraction: the H stats use raw es with an eb / eb^2
    post-scale on the tiny [M, S, 6] aggregate, and the m-contractions
    (softmax normalizer z, prediction dot) absorb eb into their per-(t,m)
    right-hand-side vectors.
  - each gathered chunk's es half is PE-transposed (f16 PSUM staging +
    scalar-engine copy) into a resident esT with m on partitions;
    z[b,t] = sum_m es*eb and num[b,t] = sum_m es*(eb*g) are then per-t PE
    matvecs (lhsT = esT_t, rhs = eb / eb*g columns) - no broadcasts, no
    big vector reduces, no DMA transpose.
  - per-(b,t) evidence reduces to THREE [S,M] batch sums via tiny per-t
    PE matmuls (contraction over the 128 batch rows on partitions): the
    rank-4 evidence combination folds into ONE host-precomputed feature
    (everything downstream is linear), and the 1/Z softmax normalizations
    fold into the per-t feature matrix.
  - two fp16 AllReduces over [S/2, M, 3] stat halves - the first overlaps
    the remaining gathers, and the prefix-sum structure lets the whole
    first half of the epilogue (cumsums, read vector g, prediction
    matvecs) run under phase A too; exclusive cumsums via
    tensor_tensor_scan with cross-half initial-value chaining;
    preds = zr * (es . (eb*g)) + pred_b.

Sharding: batch 1024 -> 128 rows per core across 8 cores (data parallel),
as the sharding hint suggests; the all-reduce is the per-slot aggregated
evidence/precision, shrunk by the rank-4 trick.
"""

import numpy as np
from contextlib import ExitStack

import concourse.bass as bass
import concourse.tile as tile
from concourse import bacc, mybir
from concourse.bass_utils import run_bass_kernel_spmd

# problem dims (hardcoded per spec)
B, S, M, K, V, E, NQ, C = 1024, 512, 64, 64, 128, 64, 10000, 4
NCORES = 8
BL = B // NCORES            # 128 batch rows per core
TC = 32                     # timestep chunk
NCH = S // TC               # 16 chunks
NIDX = BL * TC              # gather indices per chunk = 4096
# tapered chunk schedule: short first chunk (earlier first gather), short
# last chunks (shorter post-gather serial tail)
CHUNKS = [16, 16] + [32] * 15
NSEG = 2                    # stat all-reduce segments (collectives)
SEG_ENDS = [256, 512]       # segment boundaries (chunk-aligned)
M2 = 2 * M                  # packed row: [es | es^2]
F32 = mybir.dt.float32
F16 = mybir.dt.float16
I16 = mybir.dt.int16
GSCALE = 1024.0            # fp16 pre-scale for the tiny g values
ALU = mybir.AluOpType
AXT = mybir.AxisListType
ACTF = mybir.ActivationFunctionType

_CACHE = {}


def _build(single_core=False, reps=1):
    """Build the Bass program once per process.

    reps > 1 unrolls the whole algorithm multiple times (used only for
    wall-clock slope timing; outputs are just overwritten).
    """
    nc = bacc.Bacc("TRN2", target_bir_lowering=False, debug=False,
                   num_devices=1 if single_core else NCORES)

    t_esim = nc.dram_tensor("esim", [NQ + 1, M2], F16, kind="ExternalInput")
    t_idx = nc.dram_tensor("idx16", [128, NCH * NIDX // 16], I16,
                           kind="ExternalInput")
    t_ftab = nc.dram_tensor("ftab", [BL, S, 2], F16, kind="ExternalInput")
    t_ebdup = nc.dram_tensor("ebdup", [128, S], F16, kind="ExternalInput")
    t_ebm = nc.dram_tensor("ebm", [M, S], F16, kind="ExternalInput")
    t_ebm2 = nc.dram_tensor("ebm2", [M, S], F16, kind="ExternalInput")
    t_ident = nc.dram_tensor("ident", [128, 128], F16, kind="ExternalInput")
    t_dup = nc.dram_tensor("dup64", [M, 128], F16, kind="ExternalInput")
    t_sc = nc.dram_tensor("scal", [M, 8], F32, kind="ExternalInput")
    t_pb = nc.dram_tensor("pb", [BL, 1], F32, kind="ExternalInput")
    t_preds = nc.dram_tensor("preds", [BL, S], F32, kind="ExternalOutput")

    with tile.TileContext(nc) as tc, ExitStack() as outer_ctx:
        for _rep in range(reps):
            _build_body(nc, tc, outer_ctx, single_core, t_esim, t_idx,
                        t_ftab, t_ebdup, t_ebm, t_ebm2, t_ident, t_dup,
                        t_sc, t_pb, t_preds)
    nc.compile()
    return nc


def _build_body(nc, tc, _outer, single_core, t_esim, t_idx, t_ftab, t_ebdup,
                t_ebm, t_ebm2, t_ident, t_dup, t_sc, t_pb, t_preds):
    with ExitStack() as ctx:
        cpool = ctx.enter_context(tc.tile_pool(name="const", bufs=1))
        dpool = ctx.enter_context(tc.tile_pool(name="dram", bufs=1,
                                               space="DRAM"))
        # internal DRAM (quarters, so the early AllReduces overlap phase A)
        seg0 = [0] + SEG_ENDS[:-1]
        d_hin = [dpool.tile([M, (SEG_ENDS[h] - seg0[h]) * 3], F16,
                            name=f"d_hin{h}") for h in range(NSEG)]
        d_hout = [dpool.tile([M, (SEG_ENDS[h] - seg0[h]) * 3], F16,
                             addr_space="Shared", name=f"d_hout{h}")
                  for h in range(NSEG)]

        # resident SBUF; chunk-0 gather indices land first so the first
        # gather starts as early as possible
        I0 = BL * CHUNKS[0] // 16
        idx_sb = cpool.tile([128, NCH * NIDX // 16], I16)
        nc.sync.dma_start(idx_sb[:, 0:I0], t_idx.ap()[:, 0:I0])
        nc.sync.dma_start(idx_sb[:, I0:], t_idx.ap()[:, I0:])
        ftab_sb = cpool.tile([BL, S, 2], F16)
        nc.sync.dma_start(ftab_sb[:], t_ftab.ap())
        ebdup_sb = cpool.tile([128, S], F16)
        nc.sync.dma_start(ebdup_sb[:], t_ebdup.ap())
        ebm_sb = cpool.tile([M, S], F16)
        nc.sync.dma_start(ebm_sb[:], t_ebm.ap())
        ebm2_sb = cpool.tile([M, S], F16)
        nc.sync.dma_start(ebm2_sb[:], t_ebm2.ap())
        ident_sb = cpool.tile([128, 128], F16)
        nc.sync.dma_start(ident_sb[:], t_ident.ap())
        dup_sb = cpool.tile([M, 128], F16)
        nc.sync.dma_start(dup_sb[:], t_dup.ap())
        sc_sb = cpool.tile([M, 8], F32)
        nc.sync.dma_start(sc_sb[:], t_sc.ap())
        pb_sb = cpool.tile([BL, 1], F32)
        nc.sync.dma_start(pb_sb[:], t_pb.ap())
        zr = cpool.tile([BL, S], F32)
        hbuf = cpool.tile([M, S * 3], F16)
        # transposed es, resident: esT[(t%2)*64 + m, t//2, b] = es[b, t, m]
        esT = cpool.tile([128, S // 2, 128], F16)

        # ---------------- phase A: gather [es|es^2], PE-transpose es,
        # z matvecs, softmax stats + H matmuls.  Tapered chunk schedule:
        # small first chunk so the first gather starts sooner, small last
        # chunks so the post-gather serial tail is short.
        with ExitStack() as actx:
            apool = actx.enter_context(tc.tile_pool(name="pha", bufs=1))
            spool = actx.enter_context(tc.tile_pool(name="phs", bufs=2))
            pspool = actx.enter_context(
                tc.tile_pool(name="php", bufs=2, space="PSUM"))
            ptpool = actx.enter_context(
                tc.tile_pool(name="pht", bufs=1, space="PSUM"))
            t0 = 0
            for c, tcs in enumerate(CHUNKS):
                ts = slice(t0, t0 + tcs)
                nidx = BL * tcs
                i0 = t0 * BL // 16
                getag, gebufs = ("ge32", 5) if tcs == 32 else ("ge16", 2)
                ge = apool.tile([BL, tcs * M2], F16, tag=getag, bufs=gebufs)
                ge3 = ge[:].rearrange("p (a b) -> p a b", b=M2)
                # the last chunk's gather is split in two so its compute
                # pipeline starts halfway through (shorter serial tail)
                ngath = 2 if c == len(CHUNKS) - 1 else 1
                for gs in range(ngath):
                    nsub = nidx // ngath
                    tsub = tcs // ngath
                    nc.gpsimd.dma_gather(
                        out_ap=ge3[:, gs * tsub:(gs + 1) * tsub, :],
                        in_ap=t_esim.ap(),
                        idxs_ap=idx_sb[:, i0 + gs * nsub // 16:
                                       i0 + (gs + 1) * nsub // 16],
                        num_idxs=nsub,
                        num_idxs_reg=nsub,
                        elem_size=M2,
                        single_packet=False,
                    )
                es3 = ge3[:, :, 0:M]
                es23 = ge3[:, :, M:M2]
                # PE-transpose the es half into f16 PSUM (16 timesteps per
                # 1-bank staging tile), then scalar engine copies to the
                # resident esT (keeps the DMA engines free for the gathers)
                for sb in range(tcs // 16):
                    pst = ptpool.tile([128, 8, 128], F16, tag="pst", bufs=2)
                    for tt in range(16):
                        t = sb * 16 + tt
                        nc.tensor.transpose(
                            pst[(t % 2) * M:(t % 2 + 1) * M, tt // 2, :],
                            ge3[:, t, 0:M], ident_sb[:])
                    nc.scalar.copy(
                        esT[:, (t0 + sb * 16) // 2:(t0 + sb * 16) // 2 + 8,
                            :], pst[:])
                # z[b,t] = sum_m es * eb  via per-t matvecs on the PE;
                # zp and hp share one 1-bank PSUM tile.  The LAST chunk
                # processes z/features/H in 16-step sub-blocks so the
                # final AllReduce can start sooner.
                zh = pspool.tile([128, 128], F32, tag="zh")
                zp = zh[:, 0:tcs]
                hp = zh[0:M, 32:32 + tcs * 3]
                zr2 = spool.tile([BL, tcs], F32, tag=f"zr2{tcs}")
                fp = spool.tile([BL, tcs, 3], F16, tag=f"fp{tcs}")
                sub = 16 if c == len(CHUNKS) - 1 else tcs
                for s0 in range(0, tcs, sub):
                    sl = slice(s0, s0 + sub)
                    tsl = slice(t0 + s0, t0 + s0 + sub)
                    for t in range(s0, s0 + sub):
                        tg = t0 + t
                        half = tg % 2
                        nc.tensor.matmul(
                            zp[:, t:t + 1],
                            lhsT=esT[half * M:(half + 1) * M, tg // 2, :],
                            rhs=ebdup_sb[half * M:(half + 1) * M,
                                         tg:tg + 1],
                            start=True, stop=True)
                    zrc = zr[:, tsl]
                    nc.vector.reciprocal(zrc, zp[:, sl])
                    nc.vector.tensor_tensor(zr2[:, sl], zrc, zrc, ALU.mult)
                    # per-t feature matrix: [zr, pr*zr, F0*zr2] (F0 combines
                    # the four es^2 features on the host - everything
                    # downstream of the H stats is linear in them)
                    nc.vector.tensor_copy(fp[:, sl, 0:1], zrc.unsqueeze(2))
                    nc.vector.tensor_tensor(fp[:, sl, 1:2],
                                            ftab_sb[:, tsl, 0:1],
                                            zrc.unsqueeze(2), ALU.mult)
                    nc.vector.tensor_tensor(fp[:, sl, 2:3],
                                            ftab_sb[:, tsl, 1:2],
                                            zr2[:, sl].unsqueeze(2),
                                            ALU.mult)
                    # H stats: contraction over batch on the PE, per t
                    for t in range(s0, s0 + sub):
                        nc.tensor.matmul(hp[:, t * 3:t * 3 + 2],
                                         lhsT=es3[:, t, :],
                                         rhs=fp[:, t, 0:2], start=True,
                                         stop=True)
                        nc.tensor.matmul(hp[:, t * 3 + 2:t * 3 + 3],
                                         lhsT=es23[:, t, :],
                                         rhs=fp[:, t, 2:3], start=True,
                                         stop=True)
                    nc.vector.tensor_copy(
                        hbuf[:, (t0 + s0) * 3:(t0 + s0 + sub) * 3],
                        hp[:, s0 * 3:(s0 + sub) * 3])
                t0 += tcs
                # kick off each quarter's AllReduce as soon as its stats
                # are complete so the early ones overlap phase A
                if t0 in SEG_ENDS:
                    h = SEG_ENDS.index(t0)
                    hsl = slice(seg0[h] * 3, t0 * 3)
                    nc.sync.dma_start(d_hin[h][:], hbuf[:, hsl])
                    if single_core:
                        nc.sync.dma_start(d_hout[h][:], d_hin[h][:])
                    else:
                        nc.gpsimd.collective_compute(
                            "AllReduce", ALU.add,
                            replica_groups=[list(range(NCORES))],
                            ins=[d_hin[h][:].opt()],
                            outs=[d_hout[h][:].opt()],
                        )

        # ---------------- phases B/C/D, per S/4 quarter: the prefix-sum
        # structure means each quarter (after its AllReduce) is fully
        # computable while later chunks are still gathering, so only the
        # last quarter sits on the serial tail.  cs/(cs+1e-8) == 1 to
        # float precision for the attention column sums here, so cc
        # simplifies to P/B.
        DC = 128
        hs = cpool.tile([M, S, 3], F16)
        gdup = cpool.tile([128, S], F16)
        rtile = cpool.tile([BL, S], F32)
        cpool2 = ctx.enter_context(tc.tile_pool(name="phc", bufs=2))
        psg = ctx.enter_context(tc.tile_pool(name="phg", bufs=2,
                                             space="PSUM"))
        psd = ctx.enter_context(tc.tile_pool(name="phdp", bufs=2,
                                             space="PSUM"))
        ch_prev = ccs_prev = None
        hs_prev = 0
        for h in range(NSEG):
            HS = SEG_ENDS[h] - seg0[h]
            hsl = slice(seg0[h], SEG_ENDS[h])
            hsv = hs[:, hsl, :]
            nc.sync.dma_start(hsv, d_hout[h][:])
            # sfac2 = cc/(cs*B)*eb^2 algebraically reduces to
            # hs1*ebm^2/(B*hs0), so hck = (hs2*hs1) * (1/hs0) * (ebm^2/B)
            # (ebm2_sb holds ebm^2/B)
            r0 = cpool2.tile([M, HS], F32, tag="r0")
            nc.vector.reciprocal(r0[:], hsv[:, :, 0])
            cc = cpool2.tile([M, HS], F32, tag="cc")
            nc.vector.scalar_tensor_tensor(cc[:], hsv[:, :, 1], 1.0 / B,
                                           ebm_sb[:, hsl], ALU.mult,
                                           ALU.mult)
            # num = cumsum_excl(Hc * sfac2) + n0pw
            hck = cpool2.tile([M, HS], F32, tag="hck")
            nc.vector.tensor_tensor(hck[:], hsv[:, :, 2], hsv[:, :, 1],
                                    ALU.mult)
            nc.vector.tensor_tensor(hck[:], hck[:], r0[:], ALU.mult)
            nc.vector.tensor_tensor(hck[:], hck[:], ebm2_sb[:, hsl],
                                    ALU.mult)
            ch = cpool2.tile([M, HS + 1], F32, tag="ch")
            if h == 0:
                nc.vector.memset(ch[:, 0:1], 0.0)
                ini_h = 0.0
            else:
                nc.vector.tensor_copy(ch[:, 0:1],
                                      ch_prev[:, hs_prev:hs_prev + 1])
                ini_h = ch_prev[:, hs_prev:hs_prev + 1]
            nc.vector.tensor_tensor_scan(ch[:, 1:HS + 1], hck[:], hck[:],
                                         ini_h, ALU.add, ALU.bypass)
            num = cpool2.tile([M, HS], F32, tag="num")
            nc.vector.tensor_scalar_add(num[:], ch[:, 0:HS], sc_sb[:, 5:6])
            # den = alo + CC_excl ; g = num / den
            ccs = cpool2.tile([M, HS + 1], F32, tag="ccs")
            if h == 0:
                nc.vector.memset(ccs[:, 0:1], 0.0)
                ini_c = 0.0
            else:
                nc.vector.tensor_copy(ccs[:, 0:1],
                                      ccs_prev[:, hs_prev:hs_prev + 1])
                ini_c = ccs_prev[:, hs_prev:hs_prev + 1]
            nc.vector.tensor_tensor_scan(ccs[:, 1:HS + 1], cc[:], cc[:],
                                         ini_c, ALU.add, ALU.bypass)
            den = cpool2.tile([M, HS], F32, tag="den")
            nc.vector.tensor_scalar_add(den[:], ccs[:, 0:HS], sc_sb[:, 4:5])
            rden = cpool2.tile([M, HS], F32, tag="rden")
            nc.vector.reciprocal(rden[:], den[:])
            g = cpool2.tile([M, HS], F32, tag="g")
            nc.vector.tensor_tensor(g[:], num[:], rden[:], ALU.mult)
            ch_prev, ccs_prev, hs_prev = ch, ccs, HS
            # gamma = g * eb * GSCALE (fp16 normal range), duplicated onto
            # both partition halves with a tiny PE matmul
            # (dup_sb[k, i] = [i%64==k])
            gm16 = cpool2.tile([M, HS], F16, tag="gm16")
            nc.vector.scalar_tensor_tensor(gm16[:], g[:], GSCALE,
                                           ebm_sb[:, hsl], ALU.mult,
                                           ALU.mult)
            gp_ps = psg.tile([128, HS], F32, tag="gp_ps")
            nc.tensor.matmul(gp_ps[:], lhsT=dup_sb[:], rhs=gm16[:],
                             start=True, stop=True)
            nc.scalar.copy(gdup[:, hsl], gp_ps[:])
            # phase D for this half: super-chunks of 128 timesteps (one
            # PSUM bank, one fused multiply each)
            for c in range(seg0[h] // DC, SEG_ENDS[h] // DC):
                rp = psd.tile([BL, DC], F32, tag="rp")
                for t in range(DC):
                    tg = c * DC + t
                    half = tg % 2
                    nc.tensor.matmul(
                        rp[:, t:t + 1],
                        lhsT=esT[half * M:(half + 1) * M, tg // 2, :],
                        rhs=gdup[half * M:(half + 1) * M, tg:tg + 1],
                        start=True, stop=True)
                # fold the PSUM read into the zr multiply
                nc.vector.tensor_tensor(rtile[:, c * DC:(c + 1) * DC],
                                        rp[:], zr[:, c * DC:(c + 1) * DC],
                                        ALU.mult)
            nc.vector.tensor_scalar(rtile[:, hsl], rtile[:, hsl],
                                    1.0 / GSCALE, pb_sb[:, 0:1], ALU.mult,
                                    ALU.add)
        # preds out-DMAs last, so they can't head-of-line block the hs
        # landings on the SP sequencer
        for h in range(NSEG):
            hsl = slice(seg0[h], SEG_ENDS[h])
            nc.sync.dma_start(t_preds.ap()[:, hsl], rtile[:, hsl])


def _softplus(x):
    return np.logaddexp(0.0, x)


def _host_prep(inputs):
    """All the cheap host-side precomputation; returns per-core in_maps."""
    q_ids = np.asarray(inputs["q_ids"], np.int64)          # [B, S]
    responses = np.asarray(inputs["responses"], np.int64)  # [B, S]
    q_table = np.asarray(inputs["q_table"], np.float32)
    key_embeds = np.asarray(inputs["key_embeds"], np.float32)
    alpha_mean = np.asarray(inputs["alpha_mean"], np.float32)
    alpha_log_var = np.asarray(inputs["alpha_log_var"], np.float32)
    beta_base = np.asarray(inputs["beta_base"], np.float32)
    beta_offsets = np.asarray(inputs["beta_offsets"], np.float32)
    theta_mean0 = np.asarray(inputs["theta_mean0"], np.float32)
    theta_log_var0 = np.asarray(inputs["theta_log_var0"], np.float32)
    q2k_w = np.asarray(inputs["q2k_w"], np.float32)
    q2k_b = np.asarray(inputs["q2k_b"], np.float32)
    qa_w = np.asarray(inputs["qa_w"], np.float32)
    qa_b = np.asarray(inputs["qa_b"], np.float32)
    qae_w = np.asarray(inputs["qae_w"], np.float32)
    qae_b = np.asarray(inputs["qae_b"], np.float32)
    pred_w = np.asarray(inputs["pred_w"], np.float32)
    pred_b = np.asarray(inputs["pred_b"], np.float32)
    alpha_noise = np.asarray(inputs["alpha_noise"], np.float32)
    beta_noise = np.asarray(inputs["beta_noise"], np.float32)

    # sim table: folds q_table @ q2k_w @ key_embeds.T (+ bias) into a
    # gather; exp is pre-applied and es^2 packed alongside so one 256B
    # descriptor brings both powers
    w_qm = q2k_w @ key_embeds.T                            # [E, M]
    b_m = q2k_b @ key_embeds.T                             # [M]
    es = np.exp(q_table @ w_qm + b_m[None]).astype(np.float32)
    esim = np.empty((NQ + 1, M2), np.float16)
    esim[:, 0:M] = es
    esim[:, M:M2] = es * es

    # per-(t, m) logit bias -> eb = exp(bias)
    alpha = np.exp(alpha_mean[None] + alpha_noise
                   * np.exp(0.5 * alpha_log_var)[None])    # [S, M]
    base = beta_base[None] + beta_noise * 0.1              # [S, M]
    offs = _softplus(beta_offsets)                         # [M, C-1]
    cum = np.concatenate([np.zeros((M, 1), np.float32),
                          np.cumsum(offs, 1)[:, :C - 2]], 1)
    beta_mean = base + cum.mean(1)[None]
    diff_sim = np.exp(-0.5 * beta_mean ** 2)
    ebt = np.exp(0.3 * alpha + 0.2 * diff_sim).astype(np.float32)  # [S, M]
    ebT = np.ascontiguousarray(ebt.T)                      # [M, S]
    ebdup = np.concatenate([ebT, ebT], axis=0).astype(np.float16)  # [128, S]
    ebm = ebT.astype(np.float16)
    ebm2 = (ebT * ebT / B).astype(np.float16)

    ident = np.eye(128, dtype=np.float16)
    dup64 = np.concatenate([np.eye(M), np.eye(M)], axis=1).astype(np.float16)

    # evidence scalars per (b, t)
    rn = responses.astype(np.float32) / (C - 1)
    p = np.clip(rn, 0.01, 0.99)
    ae = np.log(p) - np.log1p(-p)
    pr = 0.5 + np.abs(rn - 0.5) * 2.0
    q01 = q_ids.astype(np.float32) / NQ

    # rank-4 decomposition of comb over V, folded into ONE feature F0
    w0v = qa_w[0] @ qae_w
    w1v = qa_w[1] @ qae_w
    bv = qa_b @ qae_w + qae_b
    pw = pred_w[:, 0]
    gp = 0.5 * np.array([w0v @ pw, w1v @ pw, bv @ pw, pw.sum()], np.float32)
    f0 = gp[0] * q01 + gp[1] * rn + gp[2] + gp[3] * ae     # [B, S]

    alo = np.exp(-theta_log_var0[:, 0])                    # [M]
    n0pw = alo * (theta_mean0 @ pw)                        # [M]
    sc = np.zeros((M, 8), np.float32)
    sc[:, 4] = alo
    sc[:, 5] = n0pw

    pb = np.full((BL, 1), float(pred_b[0]), np.float32)

    in_maps = []
    for core in range(NCORES):
        bs = slice(core * BL, (core + 1) * BL)
        qs = q_ids[bs]                                     # [128, S]
        # gather indices, chunk-major, wrapped in 16 partitions
        blocks = []
        t0 = 0
        for tcs in CHUNKS:
            flat = qs[:, t0:t0 + tcs].T.reshape(-1)          # t-major
            w16 = flat.reshape(BL * tcs // 16, 16).T         # [16, nidx/16]
            blocks.append(np.tile(w16, (8, 1)))
            t0 += tcs
        idx16 = np.concatenate(blocks, axis=1).astype(np.int16)

        ftab = np.empty((BL, S, 2), np.float16)
        ftab[:, :, 0] = pr[bs]
        ftab[:, :, 1] = f0[bs]

        in_maps.append({
            "esim": esim,
            "idx16": idx16,
            "ftab": ftab,
            "ebdup": ebdup,
            "ebm": ebm,
            "ebm2": ebm2,
            "ident": ident,
            "dup64": dup64,
            "scal": sc,
            "pb": pb,
        })
    return in_maps


def _run(in_maps, **kw):
    if "nc" not in _CACHE:
        _CACHE["nc"] = _build()
    res = run_bass_kernel_spmd(_CACHE["nc"], in_maps,
                               core_ids=list(range(NCORES)), **kw)
    preds = np.concatenate([res.results[c]["preds"] for c in range(NCORES)],
                           axis=0)
    return preds.astype(np.float32), res


def kernel(**inputs) -> np.ndarray:
    return _run(_host_prep(inputs))[0]


if __name__ == "__main__":
    pass
